# revision 6
# baseline (speedup 1.0000x reference)
"""CTC total-loss kernel for Trainium2 (8 NeuronCores, Bass/Tile).

Strategy (data-parallel over batch, 4 examples per core):

 * The softmax denominator decouples from the CTC alpha recursion in the
   probability domain:  loss_b = -log(l1u + l2u) + sum_{t<al} lse[t,b] - tilt,
   where l*u come from an UNNORMALIZED recursion over exp(acts at lattice
   labels).  So each core runs two independent pipelines:
     1. stream its 33.5MB acts slab once, computing per-(t,b) sum(exp(acts))
        with a single fused ACT Exp+accum instruction per (128,4096) tile;
     2. run the alpha recursion over the per-example lattice emissions
        (33 vocab rows per example, gathered host-side during input prep;
        the TRN2 indirect-DMA engine only supports contiguous row gathers,
        so the 0.5MB/core strided label gather rides in as an input).
 * The alpha recursion is computed s-major: column s over all t is a
   first-order linear recurrence x_t = E_t * (x_{t-1} + u_t), one
   tensor_tensor_scan instruction per (column, half).  65 columns replace
   512 serial timesteps; shifts in s are free AP offsets.
 * All columns live in one persistent SBUF tile (xall); boundary readout,
   renormalization and the t=256 re-seed are strided vector ops, and the
   full column matrix is dumped to DRAM with one DMA per half.  The lse
   stream is emitted first so its big DMA loads are enqueued ahead of the
   (serial) scan chain.
 * f32 dynamic range is controlled by a per-(b,t) exponential tilt
   gamma_t = max_j(gathered acts)[t] + C_TILT, plus one renormalization of
   the boundary state at t=256 to a mid-window target.  The tilt is folded
   into the shipped emissions; corrections (cumsum of gamma, renorm factor)
   are folded back host-side in log domain (validated margins ~>8 nats to
   f32 limits on the reference input distribution).

The device program is input-independent (all data dependence flows through
input tensors), so it SPMDs across the 8 cores and compiles once.
Host work is index prep (labels -> gather indices / skip masks) and the
final ~100-flop log-domain assembly of the scalar loss.
"""

import numpy as np

import concourse.bass as bass
import concourse.bacc as bacc
import concourse.tile as tile
from concourse import mybir

F32 = mybir.dt.float32
BF16 = mybir.dt.bfloat16
I32 = mybir.dt.int32

T, B, V, LMAX = 512, 32, 4096, 32
NCORES = 8
BC = B // NCORES            # 4 examples per core
S = 2 * LMAX + 1            # 65 lattice states
J = LMAX + 1                # 33 gathered vocab slots (blank + labels)
TH = T // 2                 # 256: renorm halfway
NT = (T * BC) // 128        # 16 stream tiles of (128, V)
C_TILT = -1.20              # tilt constant on top of per-t max
TB_LOG = 58.0               # renorm boundary target: max -> e^TB_LOG
CHUNKS = 4                  # j-chunks for the E pipeline
CW = TH + 1                 # column width in xall (slot 0 = state at t-1)
NCOL = S + 2                # 2 virtual columns (s=-2, s=-1) + 65 real

_CACHE = {}


def _j_chunks():
    # split J=33 slots into CHUNKS j-aligned chunks
    base = J // CHUNKS
    sizes = [base] * CHUNKS
    for i in range(J - base * CHUNKS):
        sizes[i] += 1
    out = []
    j0 = 0
    for sz in sizes:
        out.append((j0, sz))
        j0 += sz
    return out


def _build_nc():
    nc = bacc.Bacc(None)
    acts_d = nc.dram_tensor("acts", [T, BC, V], F32, kind="ExternalInput")
    gsub_d = nc.dram_tensor("gsub", [BC, J * T], F32, kind="ExternalInput")
    skipm_d = nc.dram_tensor("skipm", [BC, S], F32, kind="ExternalInput")
    xcols_d = nc.dram_tensor("xcols", [BC, S, T], F32, kind="ExternalOutput")
    rfac_d = nc.dram_tensor("rfac", [BC, 1], F32, kind="ExternalOutput")
    sums_d = nc.dram_tensor("sums", [128, NT], F32, kind="ExternalOutput")

    acts_rows = acts_d[:].rearrange("t b v -> (t b) v")     # (2048, 4096)
    chunks = _j_chunks()

    with tile.TileContext(nc) as tc:
        with (
            tc.tile_pool(name="small", bufs=1) as small,
            tc.tile_pool(name="big", bufs=1) as big,
            tc.tile_pool(name="gload", bufs=2) as gload,
            tc.tile_pool(name="up", bufs=2) as up,
            tc.tile_pool(name="stream", bufs=3) as stream,
            tc.tile_pool(name="psum", bufs=1, space="PSUM") as psump,
        ):
            # ---------------- persistent tiles ----------------
            E = big.tile([BC, J * T], BF16)        # tilted exp(gathered)
            xall = big.tile([BC, NCOL * CW], F32)  # all columns, one half

            skipm_t = small.tile([BC, S], F32)
            nc.gpsimd.dma_start(out=skipm_t[:], in_=skipm_d[:])
            negc = small.tile([BC, 1], F32)
            nc.vector.memset(negc[:], -C_TILT)
            zbias = small.tile([128, 1], F32)
            nc.vector.memset(zbias[:], 0.0)
            bnd = small.tile([BC, S], F32)
            bsc = small.tile([BC, S], F32)
            m_t = small.tile([BC, 1], F32)
            r0_t = small.tile([BC, 1], F32)
            r_t = small.tile([BC, 1], F32)
            sums = small.tile([128, NT], F32)

            # ---------------- tilted gathered acts in -> E ----------------
            # chunked so early columns' emissions are ready sooner
            for (j0, nj) in chunks:
                gch = gload.tile([BC, max(nj for _, nj in chunks) * T], F32,
                                 tag="gch")
                nc.gpsimd.dma_start(out=gch[:, :nj * T],
                                    in_=gsub_d[:, j0 * T:(j0 + nj) * T])
                nc.scalar.activation(
                    out=E[:, j0 * T:(j0 + nj) * T], in_=gch[:, :nj * T],
                    func=mybir.ActivationFunctionType.Exp,
                    bias=negc[:], scale=1.0)

            # ---------------- lse stream (emitted early: its big DMA
            # loads must not queue behind the scan chain) ----------------
            for i in range(NT):
                xt = stream.tile([128, V], F32, tag="xt")
                nc.sync.dma_start(out=xt[:],
                                  in_=acts_rows[i * 128:(i + 1) * 128, :])
                ex = psump.tile([128, V], F32, tag="ex")
                nc.scalar.activation(
                    out=ex[:], in_=xt[:],
                    func=mybir.ActivationFunctionType.Exp,
                    bias=zbias[:], scale=1.0,
                    accum_out=sums[:, i:i + 1])
            nc.gpsimd.dma_start(out=sums_d[:], in_=sums[:])

            # ---------------- s-major scans, two halves ----------------
            def cbase(s):
                return (s + 2) * CW

            # init: zero virtual columns fully, zero slot0 of real columns,
            # then the alpha0 seed: vcol s=-1 slot0 = 1.
            nc.vector.memset(xall[:, 0:2 * CW], 0.0)
            nc.vector.memset(xall[:, 2 * CW:2 * CW + 1 + (S - 1) * CW:CW], 0.0)
            nc.vector.memset(xall[:, CW:CW + 1], 1.0)

            for h in (0, 1):
                toff = h * TH
                if h == 1:
                    # boundary state -> renorm to e^TB_LOG, reseed slot0s
                    nc.vector.tensor_copy(
                        out=bnd[:],
                        in_=xall[:, 2 * CW + TH:2 * CW + TH + 1 + (S - 1) * CW:CW])
                    nc.vector.reduce_max(out=m_t[:], in_=bnd[:],
                                         axis=mybir.AxisListType.X)
                    nc.vector.reciprocal(out=r0_t[:], in_=m_t[:])
                    nc.vector.tensor_scalar_mul(r_t[:], r0_t[:],
                                                float(np.exp(TB_LOG)))
                    nc.gpsimd.dma_start(out=rfac_d[:], in_=r_t[:])
                    nc.vector.tensor_scalar_mul(bsc[:], bnd[:], r_t[:, 0:1])
                    nc.vector.memset(xall[:, CW:CW + 1], 0.0)
                    nc.vector.tensor_copy(
                        out=xall[:, 2 * CW:2 * CW + 1 + (S - 1) * CW:CW],
                        in_=bsc[:])
                for s in range(S):
                    base = cbase(s)
                    pm1 = cbase(s - 1)
                    pm2 = cbase(s - 2)
                    j_slot = 0 if s % 2 == 0 else (s - 1) // 2 + 1
                    e_sl = E[:, j_slot * T + toff: j_slot * T + toff + TH]
                    if s % 2 == 1:
                        u = up.tile([BC, TH], F32, tag="u")
                        nc.vector.scalar_tensor_tensor(
                            out=u[:],
                            in0=xall[:, pm2:pm2 + TH],
                            scalar=skipm_t[:, s:s + 1],
                            in1=xall[:, pm1:pm1 + TH],
                            op0=mybir.AluOpType.mult,
                            op1=mybir.AluOpType.add)
                        d0 = u[:]
                    else:
                        d0 = xall[:, pm1:pm1 + TH]
                    init = 0.0 if h == 0 else bsc[:, s:s + 1]
                    nc.vector.tensor_tensor_scan(
                        out=xall[:, base + 1:base + 1 + TH],
                        data0=d0, data1=e_sl, initial=init,
                        op0=mybir.AluOpType.add, op1=mybir.AluOpType.mult)
                # one batched dump per half instead of 65 tiny DMAs: the
                # per-column dma_start issue cost (~600ns each) kept the
                # sync engine 47% busy and starved the stream loads
                xv = xall[:].rearrange("b (c w) -> b c w", w=CW)
                nc.sync.dma_start(
                    out=xcols_d[:, :, toff:toff + TH],
                    in_=xv[:, 2:, 1:1 + TH])

    nc.compile()
    return nc


def _get_nc():
    if "nc" not in _CACHE:
        _CACHE["nc"] = _build_nc()
    return _CACHE["nc"]


def host_prep(acts, labels, act_lens, label_lens):
    """Build the 8 per-core input maps."""
    acts = np.ascontiguousarray(np.asarray(acts, dtype=np.float32))
    labels = np.asarray(labels).astype(np.int64)
    al = np.asarray(act_lens).astype(np.int64)
    ll = np.asarray(label_lens).astype(np.int64)
    offsets = np.cumsum(ll) - ll
    in_maps = []
    for k in range(NCORES):
        bsl = slice(k * BC, (k + 1) * BC)
        slab = np.ascontiguousarray(acts[:, bsl, :])
        gsub = np.zeros((BC, J * T), np.float32)
        gmax = np.zeros((BC, T), np.float64)
        skipm = np.zeros((BC, S), np.float32)
        for bl in range(BC):
            b = k * BC + bl
            L = int(ll[b])
            lab = np.zeros(LMAX, np.int64)
            lab[:L] = labels[offsets[b]: offsets[b] + L]
            vs = np.concatenate([[0], lab])          # (J,)
            g = slab[:, bl, vs].astype(np.float64)   # (T, J)
            gm = g.max(axis=1)                       # (T,)
            gmax[bl] = gm
            gsub[bl] = (g - gm[:, None]).T.reshape(-1)
            skipm[bl, 1] = 1.0
            for jj in range(1, L):
                if lab[jj] != lab[jj - 1]:
                    skipm[bl, 2 * jj + 1] = 1.0
        in_maps.append({"acts": slab, "gsub": gsub, "skipm": skipm,
                        "_gmax": gmax})
    return in_maps, al, ll


def host_finalize(results, al, ll, gmaxes):
    """Assemble the scalar loss from per-core outputs."""
    total = np.float64(0.0)
    for k in range(NCORES):
        r = results[k]
        sums = np.asarray(r["sums"], np.float64)          # (128, NT)
        xcols = np.asarray(r["xcols"], np.float64)        # (BC, S, T)
        rfac = np.asarray(r["rfac"], np.float64)          # (BC, 1)
        gmax = gmaxes[k]                                  # (BC, T) f64
        lse_rows = np.log(sums.T.reshape(-1)).reshape(T, BC)
        for bl in range(BC):
            b = k * BC + bl
            L = int(ll[b])
            albb = int(al[b])
            t_star = albb - 1
            e_s = 2 * L
            rs = xcols[bl, e_s, t_star] + xcols[bl, e_s - 1, t_star]
            log_unnorm = (np.log(rs) + gmax[bl, :t_star + 1].sum()
                          + C_TILT * (t_star + 1))
            if t_star >= TH:
                log_unnorm -= np.log(rfac[bl, 0])
            loss_b = -log_unnorm + lse_rows[:albb, bl].sum()
            total += loss_b
    return np.array([total], dtype=np.float32)


def kernel(acts, labels, act_lens, label_lens):
    from concourse.bass_utils import run_bass_kernel_spmd
    in_maps, al, ll = host_prep(acts, labels, act_lens, label_lens)
    gmaxes = [m.pop("_gmax") for m in in_maps]
    nc = _get_nc()
    res = run_bass_kernel_spmd(nc, in_maps, list(range(NCORES)))
    return host_finalize(res.results, al, ll, gmaxes)



# revision 7
# speedup vs baseline: 1.1611x; 1.1611x over previous
"""CTC total-loss kernel for Trainium2 (8 NeuronCores) — wavefront scan.

Same math as the baseline (unnormalized tilted alpha recursion + separate
log-sum-exp stream), but the alpha recursion runs as a skewed WAVEFRONT:

 * Each half (TH=256 steps) is split into NB=8 time-block rows of TB=32.
   Block-row k lives in partition group 4k..4k+3 (examples b=0..3), so the
   32-lane scan instruction advances 8 block-rows at once.
 * Column s of block-row k is stored at skewed slot sigma = s+k; all cells
   of a diagonal share one free offset, so each diagonal is ONE
   stt (drive u) + ONE tensor_tensor_scan over (32, TB).
 * Block seams (state crossing t = k*TB) travel down 4 partitions via one
   stream_shuffle per diagonal into "gap cells" that directly precede each
   slot, so the stt/scan APs pick them up with no extra ops.  Partition
   group 0 (k=0) instead gets its virtual t=-1 / renormed boundary values
   from a tiny per-diagonal (4,3) copy.
 * 65 x 256 serial scan-elements per half collapse to 72 diagonals x 32,
   cutting DVE chain time ~2x.
"""

import numpy as np

import concourse.bass as bass
import concourse.bacc as bacc
import concourse.tile as tile
from concourse import mybir

F32 = mybir.dt.float32
BF16 = mybir.dt.bfloat16

T, B, V, LMAX = 512, 32, 4096, 32
NCORES = 8
BC = B // NCORES            # 4 examples per core
S = 2 * LMAX + 1            # 65 lattice states
TH = T // 2                 # 256: renorm halfway
NT = (T * BC) // 128        # 16 stream tiles of (128, V)
C_TILT = -1.20
TB_LOG = 58.0

TBW = 32                    # wavefront time-block
NB = TH // TBW              # 8 block-rows per half
NP = NB * BC                # 32 partitions
CWW = TBW + 1               # slot width incl leading gap cell
NSLOT = S + NB - 1          # 72 skewed slots per half
PADS = 3                    # pad slots before slot 0
XW = (PADS + NSLOT) * CWW   # X tile free width

_CACHE = {}


def _build_nc():
    nc = bacc.Bacc(None)
    acts_d = nc.dram_tensor("acts", [T, BC, V], F32, kind="ExternalInput")
    gskew_d = nc.dram_tensor("gskew", [2, NP, NSLOT * TBW], F32,
                             kind="ExternalInput")
    skipms_d = nc.dram_tensor("skipms", [NP, NSLOT], F32,
                              kind="ExternalInput")
    xsk_d = nc.dram_tensor("xsk", [2, NP, NSLOT * CWW], F32,
                           kind="ExternalOutput")
    rfac_d = nc.dram_tensor("rfac", [BC, 1], F32, kind="ExternalOutput")
    sums_d = nc.dram_tensor("sums", [128, NT], F32, kind="ExternalOutput")

    acts_rows = acts_d[:].rearrange("t b v -> (t b) v")     # (2048, 4096)

    def GOFF(j):
        return (PADS + j) * CWW          # gap cell of slot j

    def STO(j):
        return (PADS + j) * CWW + 1      # first data elem of slot j

    ADD = mybir.AluOpType.add
    MUL = mybir.AluOpType.mult
    mask_dn4 = [(i - 4) % 32 for i in range(32)]
    mask_up28 = [(i + 28) % 32 for i in range(32)]

    with tile.TileContext(nc) as tc:
        with (
            tc.tile_pool(name="small", bufs=1) as small,
            tc.tile_pool(name="big", bufs=1) as big,
            tc.tile_pool(name="stream", bufs=3) as stream,
            tc.tile_pool(name="psum", bufs=1, space="PSUM") as psump,
        ):
            # ---------------- persistent tiles ----------------
            E0 = big.tile([NP, NSLOT * TBW], BF16, tag="E0")
            E1 = big.tile([NP, NSLOT * TBW], BF16, tag="E1")
            g0 = big.tile([NP, NSLOT * TBW], F32, tag="g0")
            g1 = big.tile([NP, NSLOT * TBW], F32, tag="g1")
            X0 = big.tile([NP, XW], F32, tag="X0")
            X1 = big.tile([NP, XW], F32, tag="X1")
            u_t = big.tile([NP, TBW], F32, tag="u")

            skipms_t = small.tile([NP, NSLOT], F32, tag="skipms")
            negc = small.tile([NP, 1], F32, tag="negc")
            zbias = small.tile([128, 1], F32, tag="zbias")
            seedp = small.tile([BC, NSLOT + PADS], F32, tag="seedp")
            bscp = small.tile([BC, NSLOT + PADS], F32, tag="bscp")
            bndt = small.tile([NP, S], F32, tag="bndt")
            m_t = small.tile([BC, 1], F32, tag="m")
            r0_t = small.tile([BC, 1], F32, tag="r0")
            r_t = small.tile([BC, 1], F32, tag="r")
            sums = small.tile([128, NT], F32, tag="sums")

            nc.gpsimd.dma_start(out=skipms_t[:], in_=skipms_d[:])
            nc.gpsimd.dma_start(out=g0[:], in_=gskew_d[0])
            nc.gpsimd.dma_start(out=g1[:], in_=gskew_d[1])
            nc.vector.memset(negc[:], -C_TILT)
            nc.vector.memset(zbias[:], 0.0)
            # tilted exp emissions for both halves, ahead of the stream exps
            nc.scalar.activation(out=E0[:], in_=g0[:],
                                 func=mybir.ActivationFunctionType.Exp,
                                 bias=negc[:], scale=1.0)
            nc.scalar.activation(out=E1[:], in_=g1[:],
                                 func=mybir.ActivationFunctionType.Exp,
                                 bias=negc[:], scale=1.0)

            # ---------------- lse stream (DMA-roofline bound) ----------
            for i in range(NT):
                xt = stream.tile([128, V], F32, tag="xt")
                nc.sync.dma_start(out=xt[:],
                                  in_=acts_rows[i * 128:(i + 1) * 128, :])
                ex = psump.tile([128, V], F32, tag="ex")
                nc.scalar.activation(
                    out=ex[:], in_=xt[:],
                    func=mybir.ActivationFunctionType.Exp,
                    bias=zbias[:], scale=1.0,
                    accum_out=sums[:, i:i + 1])
            nc.gpsimd.dma_start(out=sums_d[:], in_=sums[:])

            # ---------------- wavefront halves ----------------
            # pad region must read as zeros (virtual x=0 for t<0 / s<0)
            nc.vector.memset(X0[:, 0:PADS * CWW], 0.0)
            nc.vector.memset(X1[:, 0:PADS * CWW], 0.0)
            # seed constants for k=0 of half 0: gap G(j) <- [j == -1]
            nc.vector.memset(seedp[:], 0.0)
            nc.vector.memset(seedp[:, 1:2], 1.0)

            for h, (X, E) in enumerate(((X0, E0), (X1, E1))):
                fixsrc = seedp if h == 0 else bscp
                for sg in range(NSLOT):
                    # seam: rows k>=1 gaps G(sg-2..sg) <- row k-1 last cells
                    nc.vector.stream_shuffle(
                        out=X[:, GOFF(sg - 2):GOFF(sg) + 1:CWW],
                        in_=X[:, GOFF(sg - 2) - 1:GOFF(sg):CWW],
                        mask=mask_dn4)
                    # k=0 gaps: virtual t=-1 (h0) / renormed boundary (h1)
                    nc.vector.tensor_copy(
                        out=X[0:BC, GOFF(sg - 2):GOFF(sg) + 1:CWW],
                        in_=fixsrc[:, sg:sg + 3])
                    # drive u_tau = x^{s-1}_{t-1} + m_s * x^{s-2}_{t-1}
                    nc.vector.scalar_tensor_tensor(
                        out=u_t[:],
                        in0=X[:, GOFF(sg - 2):GOFF(sg - 2) + TBW],
                        scalar=skipms_t[:, sg:sg + 1],
                        in1=X[:, GOFF(sg - 1):GOFF(sg - 1) + TBW],
                        op0=MUL, op1=ADD)
                    # x_tau = (u_tau + x_{tau-1}) * E_tau
                    nc.vector.tensor_tensor_scan(
                        out=X[:, STO(sg):STO(sg) + TBW],
                        data0=u_t[:],
                        data1=E[:, sg * TBW:(sg + 1) * TBW],
                        initial=X[:, GOFF(sg):GOFF(sg) + 1],
                        op0=ADD, op1=MUL)
                nc.gpsimd.dma_start(out=xsk_d[h],
                                    in_=X[:, PADS * CWW:])
                if h == 0:
                    # boundary x^s_{255} lives on partitions 28..31 (k=7);
                    # shuffle up to partitions 0..3, renorm to e^TB_LOG
                    nc.vector.stream_shuffle(
                        out=bndt[:],
                        in_=X0[:, STO(7) + TBW - 1:STO(7 + S - 1) + TBW:CWW],
                        mask=mask_up28)
                    nc.vector.reduce_max(out=m_t[:], in_=bndt[0:BC, :],
                                         axis=mybir.AxisListType.X)
                    nc.vector.reciprocal(out=r0_t[:], in_=m_t[:])
                    nc.vector.tensor_scalar_mul(r_t[:], r0_t[:],
                                                float(np.exp(TB_LOG)))
                    nc.gpsimd.dma_start(out=rfac_d[:], in_=r_t[:])
                    nc.vector.memset(bscp[:], 0.0)
                    nc.vector.tensor_scalar_mul(
                        bscp[:, 2:2 + S], bndt[0:BC, :], r_t[:, 0:1])

    nc.compile()
    return nc


def _get_nc():
    if "nc" not in _CACHE:
        _CACHE["nc"] = _build_nc()
    return _CACHE["nc"]


def host_prep(acts, labels, act_lens, label_lens):
    """Build the 8 per-core input maps (skew-laid emissions + masks)."""
    acts = np.ascontiguousarray(np.asarray(acts, dtype=np.float32))
    labels = np.asarray(labels).astype(np.int64)
    al = np.asarray(act_lens).astype(np.int64)
    ll = np.asarray(label_lens).astype(np.int64)
    offsets = np.cumsum(ll) - ll
    in_maps = []
    for k in range(NCORES):
        bsl = slice(k * BC, (k + 1) * BC)
        slab = np.ascontiguousarray(acts[:, bsl, :])
        gmax = np.zeros((BC, T), np.float64)
        gt = np.zeros((BC, T, S), np.float32)       # tilted g per state
        skipm0 = np.zeros((BC, S), np.float32)
        for bl in range(BC):
            b = k * BC + bl
            L = int(ll[b])
            lab = np.zeros(LMAX, np.int64)
            lab[:L] = labels[offsets[b]: offsets[b] + L]
            ext = np.zeros(S, np.int64)
            ext[1::2] = lab                          # odd slots = labels
            g = slab[:, bl, ext].astype(np.float64)  # (T, S)
            gm = g.max(axis=1)
            gmax[bl] = gm
            gt[bl] = (g - gm[:, None]).astype(np.float32)
            skipm0[bl, 1] = 1.0
            for jj in range(1, L):
                if lab[jj] != lab[jj - 1]:
                    skipm0[bl, 2 * jj + 1] = 1.0
        # skew layouts
        gskew = np.full((2, NB, BC, NSLOT, TBW), -1e30, np.float32)
        skipms = np.zeros((NB, BC, NSLOT), np.float32)
        for kk in range(NB):
            for h in range(2):
                blk = gt[:, h * TH + kk * TBW:h * TH + (kk + 1) * TBW, :]
                gskew[h, kk, :, kk:kk + S, :] = blk.transpose(0, 2, 1)
            skipms[kk, :, kk:kk + S] = skipm0
        in_maps.append({
            "acts": slab,
            "gskew": gskew.reshape(2, NP, NSLOT * TBW),
            "skipms": skipms.reshape(NP, NSLOT),
            "_gmax": gmax,
        })
    return in_maps, al, ll


def host_finalize(results, al, ll, gmaxes):
    total = np.float64(0.0)
    for k in range(NCORES):
        r = results[k]
        sums = np.asarray(r["sums"], np.float64)          # (128, NT)
        xsk = np.asarray(r["xsk"], np.float64)            # (2, NP, NSLOT*CWW)
        rfac = np.asarray(r["rfac"], np.float64)          # (BC, 1)
        gmax = gmaxes[k]
        lse_rows = np.log(sums.T.reshape(-1)).reshape(T, BC)
        for bl in range(BC):
            b = k * BC + bl
            L = int(ll[b])
            albb = int(al[b])
            t_star = albb - 1
            e_s = 2 * L
            h = t_star // TH
            kk = (t_star % TH) // TBW
            tau = t_star % TBW

            def val(s):
                return xsk[h, 4 * kk + bl, (s + kk) * CWW + 1 + tau]

            rs = val(e_s) + val(e_s - 1)
            log_unnorm = (np.log(rs) + gmax[bl, :t_star + 1].sum()
                          + C_TILT * (t_star + 1))
            if h == 1:
                log_unnorm -= np.log(rfac[bl, 0])
            loss_b = -log_unnorm + lse_rows[:albb, bl].sum()
            total += loss_b
    return np.array([total], dtype=np.float32)


def kernel(acts, labels, act_lens, label_lens):
    from concourse.bass_utils import run_bass_kernel_spmd
    in_maps, al, ll = host_prep(acts, labels, act_lens, label_lens)
    gmaxes = [m.pop("_gmax") for m in in_maps]
    nc = _get_nc()
    res = run_bass_kernel_spmd(nc, in_maps, list(range(NCORES)))
    return host_finalize(res.results, al, ll, gmaxes)
